# revision 1
# baseline (speedup 1.0000x reference)
"""Trainium2 Bass kernel for a 2-layer LSTM decoder (B=128, T=32, F=2048,
E=512, H=1024, V=10000), tensor-parallel over the hidden dim across 8
NeuronCores.

Sharding: core c owns hidden slice [c*128, (c+1)*128) of BOTH layers (gates
i,f,o,g for that slice = 512 gate rows per weight matrix) and vocab slice
[c*1250, (c+1)*1250) of the FC head. Full batch B=128 on every core, so
every recurrence matmul runs at full 128-wide PE utilization; the per-step
cost is one 64KB AllGather of the {h0T | h1T} slice pair (DRAM-bounce
collective, ~7us ncfw floor) whose latency is hidden behind the FC matmuls
of older timesteps.

All gate math is "orientation B" (transposed): gatesT[g, b] tiles with the
gate index on partitions, so h-slices come out of the elementwise tail
already transposed for the next step's lhsT and for the AllGather, and the
gate biases fold into the ACT bias operand (no bias matmuls).

AG payload rows are 512B ({h0T | h1T} side by side per partition) so the
readback is 512B-descriptor DMA, and a small batch of throwaway matmuls at
the end of each iteration keeps the PE HAM clock-gate at 2.4 GHz across the
collective wait.

Per-step dependency chain (iteration t emits):
  readback R of AG_{t-1}={h0(t-1), h1(t-2)} -> L0(t) -> h0T(t)
  L1(t-1) from R -> h1T(t-1);  AG_t = {h0T(t), h1T(t-1)}
  FC(t-2) from R part1 (per-step PE filler that covers AG latency)
"""

import numpy as np

import concourse.bass as bass
import concourse.mybir as mybir
from concourse import bacc
from concourse.bass_utils import run_bass_kernel_spmd
from concourse.masks import make_identity
from concourse.tile import TileContext

P = 128
NCORES = 8
B, T, F, E, H, L, V = 128, 32, 2048, 512, 1024, 2, 10000
G = 4 * H
TB = T * B                 # 4096 (t, b) rows
Hc = H // NCORES           # 128 hidden units per core
Gc = 4 * Hc                # 512 local gate rows
Vc = V // NCORES           # 1250 vocab cols per core
KE, KF, KH = E // P, F // P, H // P   # 4, 16, 8
NDUMMY = 0                # PE warm-keeper matmuls per iteration
F16 = mybir.dt.float16
F32 = mybir.dt.float32

_cache = {}

SIG = mybir.ActivationFunctionType.Sigmoid
TANH = mybir.ActivationFunctionType.Tanh


def _build_nc():
    nc = bacc.Bacc("TRN2", target_bir_lowering=False, debug=False,
                   enable_asserts=False, num_devices=NCORES)

    def din(name, shape, dt=F16):
        return nc.dram_tensor(name, shape, dt, kind="ExternalInput").ap()

    featT = din("featT", [F, B])
    emb_idx = din("emb_idx", [TB, 1], mybir.dt.int32)
    table = din("table", [V, E])
    initw = din("initw", [F, 4 * P])      # cols: h_l0 | h_l1 | c_l0 | c_l1
    initbT = din("initbT", [P, 4], F32)
    wih0T = din("wih0T", [E, Gc])
    whh0T = din("whh0T", [H, Gc])
    wih1T = din("wih1T", [H, Gc])
    whh1T = din("whh1T", [H, Gc])
    b0T = din("b0T", [P, 4], F32)
    b1row = din("b1row", [1, Gc])
    fcwT = din("fcwT", [H, Vc])
    fcb_rep = din("fcb_rep", [P, Vc], F32)

    out = nc.dram_tensor("out", [TB, Vc], F32, kind="ExternalOutput").ap()

    featT_v = featT.rearrange("(k p) b -> p k b", p=P)
    initw_v = initw.rearrange("(k p) n -> p k n", p=P)
    wih0T_v = wih0T.rearrange("(k p) g -> p k g", p=P)
    whh0T_v = whh0T.rearrange("(k p) g -> p k g", p=P)
    wih1T_v = wih1T.rearrange("(k p) g -> p k g", p=P)
    whh1T_v = whh1T.rearrange("(k p) g -> p k g", p=P)
    fcwT_v = fcwT.rearrange("(k p) v -> p k v", p=P)
    idx_v = emb_idx.rearrange("(g p) one -> p g one", p=P)

    RG = [list(range(NCORES))]

    with TileContext(nc) as tc:
        if True:
            constp = tc.alloc_tile_pool(name="const", bufs=1)
            wresp = tc.alloc_tile_pool(name="wres", bufs=1)
            statep = tc.alloc_tile_pool(name="state", bufs=1)
            x0p = tc.alloc_tile_pool(name="x0", bufs=1)
            rowsp = tc.alloc_tile_pool(name="rows", bufs=32)
            rbp = tc.alloc_tile_pool(name="rb", bufs=2)
            hpairp = tc.alloc_tile_pool(name="hpair", bufs=2)
            gactp = tc.alloc_tile_pool(name="gact", bufs=2)
            fcoutp = tc.alloc_tile_pool(name="fcout", bufs=3)
            aginp = tc.alloc_tile_pool(name="agin", bufs=2, space="DRAM")
            agoutp = tc.alloc_tile_pool(name="agout", bufs=2, space="DRAM")
            g0ps = tc.alloc_tile_pool(name="g0psum", bufs=1, space="PSUM")
            g1ps = tc.alloc_tile_pool(name="g1psum", bufs=1, space="PSUM")
            fcps = tc.alloc_tile_pool(name="fcpsum", bufs=2, space="PSUM")
            dups = tc.alloc_tile_pool(name="dumpsum", bufs=1, space="PSUM")

            # critical-path pre-loop loads first on the sync ring
            featp = tc.alloc_tile_pool(name="feat", bufs=1)
            initwp = tc.alloc_tile_pool(name="initw", bufs=1)
            featT_s = featp.tile([P, KF, B], F16)
            nc.sync.dma_start(featT_s, featT_v)
            initw_s = initwp.tile([P, KF, 4 * P], F16)
            nc.sync.dma_start(initw_s, initw_v)

            id128 = constp.tile([P, P], F16)
            make_identity(nc, id128)
            b0T_s = constp.tile([P, 4], F32, tag="b0T")
            nc.sync.dma_start(b0T_s, b0T)
            b1row_s = constp.tile([1, Gc], F16, tag="b1row")
            nc.sync.dma_start(b1row_s, b1row)
            ones1 = constp.tile([1, B], F16, tag="ones1")
            nc.gpsimd.memset(ones1, 1.0)
            initbT_s = constp.tile([P, 4], F32, tag="ibT")
            nc.sync.dma_start(initbT_s, initbT)
            idx_s = constp.tile([P, T, 1], mybir.dt.int32, tag="idx")
            nc.sync.dma_start(idx_s, idx_v)

            # ---- resident weights (scalar-engine DMA ring) ---------------
            whh0_s = wresp.tile([P, KH, Gc], F16, tag="whh0")
            nc.scalar.dma_start(whh0_s, whh0T_v)
            wih1_s = wresp.tile([P, KH, Gc], F16, tag="wih1")
            nc.scalar.dma_start(wih1_s, wih1T_v)
            whh1_s = wresp.tile([P, KH, Gc], F16, tag="whh1")
            nc.scalar.dma_start(whh1_s, whh1T_v)
            wih0_s = wresp.tile([P, KE, Gc], F16, tag="wih0")
            nc.scalar.dma_start(wih0_s, wih0T_v)
            fcw_s = wresp.tile([P, KH, Vc], F16, tag="fcw")
            nc.scalar.dma_start(fcw_s, fcwT_v)
            fcb_s = wresp.tile([P, Vc], F32, tag="fcb")
            nc.scalar.dma_start(fcb_s, fcb_rep)

            # persistent state
            c0T_s = statep.tile([P, B], F32, tag="c0")
            c1T_s = statep.tile([P, B], F32, tag="c1")

            x0T_s = x0p.tile([P, 4, T, B], F16)
            dum_ps = dups.tile([1, 512], F32)

            # ---- phase A-gather helper: Q7 desc-gen is ~1.1us per call,
            # so gathers are interleaved between collective emissions to
            # keep them off the doorbell path
            row_tiles = []

            def gather(m):
                assert m == len(row_tiles)
                rows = rowsp.tile([P, E], F16, tag="rows")
                nc.gpsimd.indirect_dma_start(
                    out=rows[:],
                    out_offset=None,
                    in_=table[:],
                    in_offset=bass.IndirectOffsetOnAxis(ap=idx_s[:, m, :], axis=0),
                )
                row_tiles.append(rows)

            for m in range(16):
                gather(m)

            # ---- phase B: h/c init (linear head, orientation B) ----------
            initps = tc.alloc_tile_pool(name="initpsum", bufs=1, space="PSUM")
            ips = initps.tile([P, 4, P], F32)
            for m in range(4):
                for k in range(KF):
                    nc.tensor.matmul(
                        ips[:, m, :],
                        initw_s[:, k, m * P : (m + 1) * P],
                        featT_s[:, k, :],
                        start=(k == 0),
                        stop=(k == KF - 1),
                    )
            hpair_init = hpairp.tile([P, 2 * B], F16, tag="hpair")
            nc.vector.tensor_scalar_add(hpair_init[:, 0:B], ips[:, 0, :],
                                        initbT_s[:, 0:1])
            nc.vector.tensor_scalar_add(hpair_init[:, B : 2 * B], ips[:, 1, :],
                                        initbT_s[:, 1:2])
            nc.vector.tensor_scalar_add(c0T_s, ips[:, 2, :],
                                        initbT_s[:, 2:3])
            nc.vector.tensor_scalar_add(c1T_s, ips[:, 3, :],
                                        initbT_s[:, 3:4])

            # ---- AG_init = {h0_init, h1_init} ----------------------------
            agin_t = aginp.tile([P, 2 * B], F16, tag="agin")
            agout_t = agoutp.tile([NCORES * P, 2 * B], F16, tag="agout",
                                  addr_space="Shared")
            nc.sync.dma_start(agin_t, hpair_init)
            nc.gpsimd.collective_compute(
                "AllGather", mybir.AluOpType.bypass, replica_groups=RG,
                ins=[agin_t[:].opt()], outs=[agout_t[:].opt()],
            )
            agout_prev = agout_t

            for m in range(16, T):
                gather(m)

            # ---- phase A-compute: embT transpose + X0T for chunk m -------
            initps.release()
            embps = tc.alloc_tile_pool(name="embpsum", bufs=2, space="PSUM")
            x0ps = tc.alloc_tile_pool(name="x0psum", bufs=1, space="PSUM")
            embcp = tc.alloc_tile_pool(name="embc", bufs=2)

            def a_compute(m):
                pt = embps.tile([P, KE, P], F16, tag="pt")
                for ke in range(KE):
                    nc.tensor.transpose(
                        pt[:, ke, :],
                        row_tiles[m][:, ke * P : (ke + 1) * P],
                        id128,
                    )
                embc = embcp.tile([P, KE, P], F16, tag="embc")
                nc.vector.tensor_copy(embc, pt)
                xps = x0ps.tile([P, 4, P], F32, tag="xps")
                for g in range(4):
                    for k in range(KE):
                        nc.tensor.matmul(
                            xps[:, g, :],
                            wih0_s[:, k, g * P : (g + 1) * P],
                            embc[:, k, :],
                            start=(k == 0),
                            stop=(k == KE - 1),
                        )
                for g in range(4):
                    nc.vector.tensor_scalar_add(
                        x0T_s[:, g, m, :], xps[:, g, :], b0T_s[:, g : g + 1]
                    )

            for m in range(4):
                a_compute(m)

            # ---- gate drains (orientation B; biases already in psum) -----
            def drain(gps, cT_s, out_hT):
                """gatesT [P,4,B] psum -> hT f16 into out_hT; updates cT_s."""
                sig_ifo = gactp.tile([P, 3, B], F32, tag="sig_ifo")
                nc.scalar.activation(sig_ifo, gps[:, 0:3, :], SIG)
                tanh_g = gactp.tile([P, B], F32, tag="tanh_g")
                nc.scalar.activation(tanh_g, gps[:, 3, :], TANH)
                nc.vector.tensor_mul(cT_s, sig_ifo[:, 1, :], cT_s)
                nc.vector.tensor_mul(tanh_g, sig_ifo[:, 0, :], tanh_g)
                nc.vector.tensor_add(cT_s, cT_s, tanh_g)
                tanh_c = gactp.tile([P, B], F32, tag="tanh_c")
                nc.scalar.activation(tanh_c, cT_s, TANH)
                nc.vector.tensor_mul(out_hT, sig_ifo[:, 2, :], tanh_c)

            # ---- main loop -----------------------------------------------
            hpair_prev = hpair_init
            for t in range(T + 2):
                # readback of AG_{t-1} (for t==0: AG_init)
                rbv = agout_prev[:].rearrange("(c p) x -> p c x", c=NCORES)
                R = rbp.tile([P, NCORES, 2 * B], F16, tag="R")
                nc.sync.dma_start(R[:, 0:4, :], rbv[:, 0:4, :])
                nc.scalar.dma_start(R[:, 4:NCORES, :], rbv[:, 4:NCORES, :])

                hpair_t = hpairp.tile([P, 2 * B], F16, tag="hpair")
                agin_t = agout_t = None
                if t <= T:
                    agin_t = aginp.tile([P, 2 * B], F16, tag="agin")
                    agout_t = agoutp.tile([NCORES * P, 2 * B], F16,
                                          tag="agout", addr_space="Shared")

                # ---- L0(t): gates0T = whh0.h0(t-1) + X0T[t] --------------
                if t < T:
                    gps0 = g0ps.tile([P, 4, B], F32, tag="g0")
                    for g in range(4):
                        for k in range(KH):
                            nc.tensor.matmul(
                                gps0[:, g, :],
                                whh0_s[:, k, g * P : (g + 1) * P],
                                R[:, k, 0:B],
                                start=(k == 0),
                                stop=False,
                            )
                        nc.tensor.matmul(
                            gps0[:, g, :],
                            id128,
                            x0T_s[:, g, t, :],
                            start=False,
                            stop=True,
                        )
                    drain(gps0, c0T_s, hpair_t[:, 0:B])
                else:
                    nc.vector.tensor_copy(hpair_t[:, 0:B], hpair_prev[:, 0:B])
                if agin_t is not None:
                    nc.scalar.dma_start(agin_t[:, 0:B], hpair_t[:, 0:B])

                # ---- L1(t-1): gates1T = wih1.h0(t-1) + whh1.h1(t-2) ------
                if 1 <= t <= T:
                    gps1 = g1ps.tile([P, 4, B], F32, tag="g1")
                    for g in range(4):
                        for k in range(KH):
                            nc.tensor.matmul(
                                gps1[:, g, :],
                                wih1_s[:, k, g * P : (g + 1) * P],
                                R[:, k, 0:B],
                                start=(k == 0),
                                stop=False,
                            )
                        for k in range(KH):
                            nc.tensor.matmul(
                                gps1[:, g, :],
                                whh1_s[:, k, g * P : (g + 1) * P],
                                R[:, k, B : 2 * B],
                                start=False,
                                stop=False,
                            )
                        nc.tensor.matmul(
                            gps1[:, g, :],
                            b1row_s[:, g * P : (g + 1) * P],
                            ones1,
                            start=False,
                            stop=True,
                        )
                    drain(gps1, c1T_s, hpair_t[:, B : 2 * B])
                elif t == 0:
                    nc.vector.tensor_copy(hpair_t[:, B : 2 * B],
                                          hpair_prev[:, B : 2 * B])

                # ---- AG_t = {h0T(t), h1T(t-1)} ---------------------------
                if t <= T:
                    nc.sync.dma_start(agin_t[:, B : 2 * B],
                                      hpair_t[:, B : 2 * B])
                    nc.gpsimd.collective_compute(
                        "AllGather", mybir.AluOpType.bypass,
                        replica_groups=RG,
                        ins=[agin_t[:].opt()], outs=[agout_t[:].opt()],
                    )
                    agout_prev = agout_t

                # ---- FC(t-2) on R part1 = h1T_full(t-2) ------------------
                if t >= 2:
                    tau = t - 2
                    for j0 in range(0, 1024, 512):
                        w = min(512, Vc - j0)
                        fps = fcps.tile([P, 512], F32, tag="fc")
                        for k in range(KH):
                            nc.tensor.matmul(
                                fps[:, :w],
                                R[:, k, B : 2 * B],
                                fcw_s[:, k, j0 : j0 + w],
                                start=(k == 0),
                                stop=(k == KH - 1),
                            )
                        ot = fcoutp.tile([P, 512], F32, tag="ot")
                        nc.vector.tensor_add(
                            ot[:, :w], fps[:, :w], fcb_s[:, j0 : j0 + w]
                        )
                        nc.scalar.dma_start(
                            out[tau * P : (tau + 1) * P, j0 : j0 + w],
                            ot[:, :w],
                        )

                # ---- A-compute filler for a later chunk ------------------
                if t + 4 < T:
                    a_compute(t + 4)

                # ---- FC tail chunk: PE filler deep in the collective wait
                if t >= 2:
                    tau = t - 2
                    j0 = 1024
                    w = Vc - j0
                    fps = fcps.tile([P, 512], F32, tag="fc")
                    for k in range(KH):
                        nc.tensor.matmul(
                            fps[:, :w],
                            R[:, k, B : 2 * B],
                            fcw_s[:, k, j0 : j0 + w],
                            start=(k == 0),
                            stop=(k == KH - 1),
                        )
                    ot = fcoutp.tile([P, 512], F32, tag="ot")
                    nc.vector.tensor_add(
                        ot[:, :w], fps[:, :w], fcb_s[:, j0 : j0 + w]
                    )
                    nc.scalar.dma_start(
                        out[tau * P : (tau + 1) * P, j0 : j0 + w],
                        ot[:, :w],
                    )

                # ---- PE warm-keepers during the collective wait ----------
                if t <= T:
                    for _ in range(NDUMMY):
                        nc.tensor.matmul(
                            dum_ps, id128[:, 0:1], whh0_s[:, 0, :],
                            start=True, stop=True,
                        )

                hpair_prev = hpair_t

            for pool in (embcp, x0ps, embps, initwp, featp, dups, fcps,
                         g1ps, g0ps, agoutp, aginp, fcoutp, gactp, hpairp,
                         rbp, rowsp, x0p, statep, wresp, constp):
                pool.release()

    nc.finalize()
    return nc


def _get_compiled():
    if "nc" not in _cache:
        _cache["nc"] = _build_nc()
    return _cache["nc"]


def _prep_inputs(features, captions, embed_table, init_h_w, init_h_b,
                 init_c_w, init_c_b, w_ih0, w_hh0, b_ih0, b_hh0,
                 w_ih1, w_hh1, b_ih1, b_hh1, fc_w, fc_b):
    f16 = lambda x: np.ascontiguousarray(np.asarray(x, dtype=np.float32)).astype(np.float16)
    f32 = lambda x: np.ascontiguousarray(np.asarray(x, dtype=np.float32))

    features = np.asarray(features, dtype=np.float32)
    captions = np.asarray(captions).astype(np.int32)

    shared = {
        "featT": f16(features.T),
        "table": f16(embed_table),
        # row r = t*B + b  ->  captions[b, t]
        "emb_idx": np.ascontiguousarray(captions.T.reshape(TB, 1)),
    }

    # torch gate order i,f,g,o -> local order [i, f, o, g]
    def gate_rows(c):
        base = np.arange(c * Hc, (c + 1) * Hc)
        return np.concatenate([base, H + base, 3 * H + base, 2 * H + base])

    def init_sel(c):
        # Linear output col r maps to (h = r // L, l = r % L)
        h_idx = np.arange(c * Hc, (c + 1) * Hc)
        return 2 * h_idx, 2 * h_idx + 1   # l0 rows, l1 rows

    in_maps = []
    for c in range(NCORES):
        rows_sel = gate_rows(c)
        l0, l1 = init_sel(c)
        ihw = np.asarray(init_h_w, np.float32)
        icw = np.asarray(init_c_w, np.float32)
        ihb = np.asarray(init_h_b, np.float32)
        icb = np.asarray(init_c_b, np.float32)
        initw = np.concatenate([ihw[l0], ihw[l1], icw[l0], icw[l1]], axis=0)
        initb = np.concatenate([ihb[l0], ihb[l1], icb[l0], icb[l1]])

        b0 = (np.asarray(b_ih0, np.float32) + np.asarray(b_hh0, np.float32))[rows_sel]
        b1 = (np.asarray(b_ih1, np.float32) + np.asarray(b_hh1, np.float32))[rows_sel]

        vsl = slice(c * Vc, (c + 1) * Vc)
        m = dict(shared)
        m.update({
            "initw": f16(initw.T),
            "initbT": f32(initb.reshape(4, P).T),
            "wih0T": f16(np.asarray(w_ih0, np.float32)[rows_sel].T),
            "whh0T": f16(np.asarray(w_hh0, np.float32)[rows_sel].T),
            "wih1T": f16(np.asarray(w_ih1, np.float32)[rows_sel].T),
            "whh1T": f16(np.asarray(w_hh1, np.float32)[rows_sel].T),
            "b0T": f32(b0.reshape(4, P).T),
            "b1row": f16(b1.reshape(1, Gc)),
            "fcwT": f16(np.asarray(fc_w, np.float32)[vsl].T),
            "fcb_rep": f32(np.broadcast_to(
                np.asarray(fc_b, np.float32)[vsl], (P, Vc))),
        })
        in_maps.append(m)
    return in_maps


last_results = None


def kernel(**inputs) -> np.ndarray:
    global last_results
    nc = _get_compiled()
    in_maps = _prep_inputs(**inputs)
    res = run_bass_kernel_spmd(nc, in_maps, core_ids=list(range(NCORES)))
    last_results = res
    parts = [res.results[c]["out"].reshape(T, B, Vc) for c in range(NCORES)]
    return np.concatenate(parts, axis=2)



# revision 3
# speedup vs baseline: 1.0997x; 1.0997x over previous
"""Trainium2 Bass kernel for a 2-layer LSTM decoder (B=128, T=32, F=2048,
E=512, H=1024, V=10000), tensor-parallel over the hidden dim across 8
NeuronCores.

Sharding: core c owns hidden slice [c*128, (c+1)*128) of BOTH layers (gates
i,g,f,o for that slice = 512 gate rows per weight matrix) and vocab slice
[c*1250, (c+1)*1250) of the FC head. Full batch B=128 on every core, so
every recurrence matmul runs at full 128-wide PE utilization.

The per-step h exchange is SPLIT into two small AllGathers so the layer-0
recurrence (the tight serial loop) does not wait for the layer-1 drain:
  AG0_t ships h0T(t) right after the L0 drain (~5us into the iteration);
  AG1_t ships h1T(t-1) after the L1 drain.
Readbacks of AG0_{t-1}/AG1_{t-1} happen at the top of iteration t on the
sync+scalar HWDGE queues.

All gate math is "orientation B" (transposed): gatesT[g, b] tiles with the
gate index on partitions. Gate biases are folded into the ACT activation
bias operand (per-partition [P,1] column of b0T/b1T), so there are no bias
matmuls and the gate drains pipeline per-gate behind the matmuls:
gate order is [i, g, f, o] so c_new and tanh(c) are ready before the o-gate
matmuls finish, leaving only sigmoid(o)+mul on the critical tail.

Per-step dependency chain (iteration t):
  readback AG0_{t-1} -> L0(t) -> h0T(t) -> AG0_t
  readback AG1_{t-1} -> L1(t-1) -> h1T(t-1) -> AG1_t
  FC(t-2) from AG1_{t-1} readback (PE filler between the drains)
"""

import numpy as np

import concourse.bass as bass
import concourse.mybir as mybir
from concourse import bacc
from concourse.bass_utils import run_bass_kernel_spmd
from concourse.masks import make_identity
from concourse.tile import TileContext

P = 128
NCORES = 8
B, T, F, E, H, L, V = 128, 32, 2048, 512, 1024, 2, 10000
G = 4 * H
TB = T * B                 # 4096 (t, b) rows
Hc = H // NCORES           # 128 hidden units per core
Gc = 4 * Hc                # 512 local gate rows
Vc = V // NCORES           # 1250 vocab cols per core
KE, KF, KH = E // P, F // P, H // P   # 4, 16, 8
F16 = mybir.dt.float16
F32 = mybir.dt.float32

_cache = {}

SIG = mybir.ActivationFunctionType.Sigmoid
TANH = mybir.ActivationFunctionType.Tanh


def _build_nc():
    nc = bacc.Bacc("TRN2", target_bir_lowering=False, debug=False,
                   enable_asserts=False, num_devices=NCORES)

    def din(name, shape, dt=F16):
        return nc.dram_tensor(name, shape, dt, kind="ExternalInput").ap()

    featT = din("featT", [F, B])
    emb_idx = din("emb_idx", [TB, 1], mybir.dt.int32)
    table = din("table", [V, E])
    initw = din("initw", [F, 4 * P])      # cols: h_l0 | h_l1 | c_l0 | c_l1
    initbT = din("initbT", [P, 4], F32)
    wih0T = din("wih0T", [E, Gc])
    whh0T = din("whh0T", [H, Gc])
    wih1T = din("wih1T", [H, Gc])
    whh1T = din("whh1T", [H, Gc])
    b0T = din("b0T", [P, 4], F32)
    b1T = din("b1T", [P, 4], F32)
    fcwT = din("fcwT", [H, Vc])
    fcb_rep = din("fcb_rep", [P, Vc], F32)

    out = nc.dram_tensor("out", [TB, Vc], F32, kind="ExternalOutput").ap()

    featT_v = featT.rearrange("(k p) b -> p k b", p=P)
    initw_v = initw.rearrange("(k p) n -> p k n", p=P)
    wih0T_v = wih0T.rearrange("(k p) g -> p k g", p=P)
    whh0T_v = whh0T.rearrange("(k p) g -> p k g", p=P)
    wih1T_v = wih1T.rearrange("(k p) g -> p k g", p=P)
    whh1T_v = whh1T.rearrange("(k p) g -> p k g", p=P)
    fcwT_v = fcwT.rearrange("(k p) v -> p k v", p=P)
    idx_v = emb_idx.rearrange("(g p) one -> p g one", p=P)

    RG = [list(range(NCORES))]

    with TileContext(nc) as tc:
        if True:
            constp = tc.alloc_tile_pool(name="const", bufs=1)
            wresp = tc.alloc_tile_pool(name="wres", bufs=1)
            statep = tc.alloc_tile_pool(name="state", bufs=1)
            x0p = tc.alloc_tile_pool(name="x0", bufs=1)
            rowsp = tc.alloc_tile_pool(name="rows", bufs=32)
            rb0p = tc.alloc_tile_pool(name="rb0", bufs=2)
            rb1p = tc.alloc_tile_pool(name="rb1", bufs=2)
            h0p = tc.alloc_tile_pool(name="h0p", bufs=2)
            h1p = tc.alloc_tile_pool(name="h1p", bufs=2)
            gactp = tc.alloc_tile_pool(name="gact", bufs=2)
            fcoutp = tc.alloc_tile_pool(name="fcout", bufs=3)
            agin0p = tc.alloc_tile_pool(name="agin0", bufs=2, space="DRAM")
            agin1p = tc.alloc_tile_pool(name="agin1", bufs=2, space="DRAM")
            agout0p = tc.alloc_tile_pool(name="agout0", bufs=2, space="DRAM")
            agout1p = tc.alloc_tile_pool(name="agout1", bufs=2, space="DRAM")
            g0ps = tc.alloc_tile_pool(name="g0psum", bufs=1, space="PSUM")
            g1ps = tc.alloc_tile_pool(name="g1psum", bufs=1, space="PSUM")
            fcps = tc.alloc_tile_pool(name="fcpsum", bufs=2, space="PSUM")

            # critical-path pre-loop loads first on the sync ring
            featp = tc.alloc_tile_pool(name="feat", bufs=1)
            initwp = tc.alloc_tile_pool(name="initw", bufs=1)
            featT_s = featp.tile([P, KF, B], F16)
            nc.sync.dma_start(featT_s, featT_v)
            initw_s = initwp.tile([P, KF, 4 * P], F16)
            nc.sync.dma_start(initw_s, initw_v)

            id128 = constp.tile([P, P], F16)
            make_identity(nc, id128)
            b0T_s = constp.tile([P, 4], F32, tag="b0T")
            nc.sync.dma_start(b0T_s, b0T)
            b1T_s = constp.tile([P, 4], F32, tag="b1T")
            nc.sync.dma_start(b1T_s, b1T)
            initbT_s = constp.tile([P, 4], F32, tag="ibT")
            nc.sync.dma_start(initbT_s, initbT)
            idx_s = constp.tile([P, T, 1], mybir.dt.int32, tag="idx")
            nc.sync.dma_start(idx_s, idx_v)

            # ---- resident weights (scalar-engine DMA ring) ---------------
            whh0_s = wresp.tile([P, KH, Gc], F16, tag="whh0")
            nc.scalar.dma_start(whh0_s, whh0T_v)
            wih1_s = wresp.tile([P, KH, Gc], F16, tag="wih1")
            nc.scalar.dma_start(wih1_s, wih1T_v)
            whh1_s = wresp.tile([P, KH, Gc], F16, tag="whh1")
            nc.scalar.dma_start(whh1_s, whh1T_v)
            wih0_s = wresp.tile([P, KE, Gc], F16, tag="wih0")
            nc.scalar.dma_start(wih0_s, wih0T_v)
            fcw_s = wresp.tile([P, KH, Vc], F16, tag="fcw")
            nc.scalar.dma_start(fcw_s, fcwT_v)
            fcb_s = wresp.tile([P, Vc], F32, tag="fcb")
            nc.scalar.dma_start(fcb_s, fcb_rep)

            # persistent state
            c0T_s = statep.tile([P, B], F32, tag="c0")
            c1T_s = statep.tile([P, B], F32, tag="c1")

            x0T_s = x0p.tile([P, 4, T, B], F16)

            # ---- phase A-gather helper -----------------------------------
            row_tiles = []

            def gather(m):
                assert m == len(row_tiles)
                rows = rowsp.tile([P, E], F16, tag="rows")
                nc.gpsimd.indirect_dma_start(
                    out=rows[:],
                    out_offset=None,
                    in_=table[:],
                    in_offset=bass.IndirectOffsetOnAxis(ap=idx_s[:, m, :], axis=0),
                )
                row_tiles.append(rows)

            for m in range(16):
                gather(m)

            # ---- phase B: h/c init (linear head, orientation B) ----------
            initps = tc.alloc_tile_pool(name="initpsum", bufs=1, space="PSUM")
            ips = initps.tile([P, 4, P], F32)
            for m in range(4):
                for k in range(KF):
                    nc.tensor.matmul(
                        ips[:, m, :],
                        initw_s[:, k, m * P : (m + 1) * P],
                        featT_s[:, k, :],
                        start=(k == 0),
                        stop=(k == KF - 1),
                    )
            h0_init = h0p.tile([P, B], F16, tag="h0")
            h1_init = h1p.tile([P, B], F16, tag="h1")
            nc.vector.tensor_scalar_add(h0_init, ips[:, 0, :], initbT_s[:, 0:1])
            nc.vector.tensor_scalar_add(h1_init, ips[:, 1, :], initbT_s[:, 1:2])
            nc.vector.tensor_scalar_add(c0T_s, ips[:, 2, :], initbT_s[:, 2:3])
            nc.vector.tensor_scalar_add(c1T_s, ips[:, 3, :], initbT_s[:, 3:4])

            # ---- AG0_init = h0_init, AG1_init = h1_init ------------------
            def emit_ag0(h_tile):
                agin_t = agin0p.tile([P, B], F16, tag="agin0")
                agout_t = agout0p.tile([NCORES * P, B], F16, tag="agout0",
                                       addr_space="Shared")
                nc.sync.dma_start(agin_t, h_tile)
                nc.gpsimd.collective_compute(
                    "AllGather", mybir.AluOpType.bypass, replica_groups=RG,
                    ins=[agin_t[:].opt()], outs=[agout_t[:].opt()],
                )
                return agout_t

            def emit_ag1(h_tile):
                agin_t = agin1p.tile([P, B], F16, tag="agin1")
                agout_t = agout1p.tile([NCORES * P, B], F16, tag="agout1",
                                       addr_space="Shared")
                nc.sync.dma_start(agin_t, h_tile)
                nc.gpsimd.collective_compute(
                    "AllGather", mybir.AluOpType.bypass, replica_groups=RG,
                    ins=[agin_t[:].opt()], outs=[agout_t[:].opt()],
                )
                return agout_t

            agout0_prev = emit_ag0(h0_init)
            agout1_prev = emit_ag1(h1_init)

            for m in range(16, T):
                gather(m)

            # ---- phase A-compute: embT transpose + X0T for chunk m -------
            initps.release()
            embps = tc.alloc_tile_pool(name="embpsum", bufs=2, space="PSUM")
            x0ps = tc.alloc_tile_pool(name="x0psum", bufs=1, space="PSUM")
            embcp = tc.alloc_tile_pool(name="embc", bufs=2)

            def a_compute(m):
                pt = embps.tile([P, KE, P], F16, tag="pt")
                for ke in range(KE):
                    nc.tensor.transpose(
                        pt[:, ke, :],
                        row_tiles[m][:, ke * P : (ke + 1) * P],
                        id128,
                    )
                embc = embcp.tile([P, KE, P], F16, tag="embc")
                nc.vector.tensor_copy(embc, pt)
                xps = x0ps.tile([P, 4, P], F32, tag="xps")
                for g in range(4):
                    for k in range(KE):
                        nc.tensor.matmul(
                            xps[:, g, :],
                            wih0_s[:, k, g * P : (g + 1) * P],
                            embc[:, k, :],
                            start=(k == 0),
                            stop=(k == KE - 1),
                        )
                nc.vector.tensor_copy(x0T_s[:, :, m, :], xps)

            for m in range(4):
                a_compute(m)

            # ---- pipelined per-gate drain (gate order i, g, f, o) --------
            def drain(gps, cT_s, out_hT, bT_s):
                sig_i = gactp.tile([P, B], F32, tag="si")
                nc.scalar.activation(sig_i, gps[:, 0, :], SIG,
                                     bias=bT_s[:, 0:1])
                tanh_g = gactp.tile([P, B], F32, tag="tg")
                nc.scalar.activation(tanh_g, gps[:, 1, :], TANH,
                                     bias=bT_s[:, 1:2])
                nc.vector.tensor_mul(tanh_g, sig_i, tanh_g)
                sig_f = gactp.tile([P, B], F32, tag="sf")
                nc.scalar.activation(sig_f, gps[:, 2, :], SIG,
                                     bias=bT_s[:, 2:3])
                nc.vector.tensor_mul(cT_s, sig_f, cT_s)
                nc.vector.tensor_add(cT_s, cT_s, tanh_g)
                tanh_c = gactp.tile([P, B], F32, tag="tc")
                nc.scalar.activation(tanh_c, cT_s, TANH)
                sig_o = gactp.tile([P, B], F32, tag="so")
                nc.scalar.activation(sig_o, gps[:, 3, :], SIG,
                                     bias=bT_s[:, 3:4])
                nc.vector.tensor_mul(out_hT, sig_o, tanh_c)

            # ---- main loop -----------------------------------------------
            for t in range(T + 2):
                # readbacks of AG0_{t-1} / AG1_{t-1}
                if t <= T:
                    R0 = rb0p.tile([P, NCORES, B], F16, tag="R0")
                    v0 = agout0_prev[:].rearrange("(c p) b -> p c b", c=NCORES)
                    nc.sync.dma_start(R0[:, 0:4, :], v0[:, 0:4, :])
                    nc.scalar.dma_start(R0[:, 4:8, :], v0[:, 4:8, :])
                if 1 <= t:
                    R1 = rb1p.tile([P, NCORES, B], F16, tag="R1")
                    v1 = agout1_prev[:].rearrange("(c p) b -> p c b", c=NCORES)
                    nc.sync.dma_start(R1[:, 0:4, :], v1[:, 0:4, :])
                    nc.scalar.dma_start(R1[:, 4:8, :], v1[:, 4:8, :])
                else:
                    R1 = None

                # ---- L0(t): gates0T = whh0.h0(t-1) + X0T[t] --------------
                if t < T:
                    gps0 = g0ps.tile([P, 4, B], F32, tag="g0")
                    for g in range(4):
                        for k in range(KH):
                            nc.tensor.matmul(
                                gps0[:, g, :],
                                whh0_s[:, k, g * P : (g + 1) * P],
                                R0[:, k, :],
                                start=(k == 0),
                                stop=False,
                            )
                        nc.tensor.matmul(
                            gps0[:, g, :],
                            id128,
                            x0T_s[:, g, t, :],
                            start=False,
                            stop=True,
                        )
                    h0t = h0p.tile([P, B], F16, tag="h0")
                    drain(gps0, c0T_s, h0t, b0T_s)
                    agout0_prev = emit_ag0(h0t)

                # ---- L1(t-1): gates1T = wih1.h0(t-1) + whh1.h1(t-2) ------
                if 1 <= t <= T:
                    gps1 = g1ps.tile([P, 4, B], F32, tag="g1")
                    for g in range(4):
                        for k in range(KH):
                            nc.tensor.matmul(
                                gps1[:, g, :],
                                wih1_s[:, k, g * P : (g + 1) * P],
                                R0[:, k, :],
                                start=(k == 0),
                                stop=False,
                            )
                        for k in range(KH):
                            nc.tensor.matmul(
                                gps1[:, g, :],
                                whh1_s[:, k, g * P : (g + 1) * P],
                                R1[:, k, :],
                                start=False,
                                stop=(k == KH - 1),
                            )
                    h1t = h1p.tile([P, B], F16, tag="h1")
                    drain(gps1, c1T_s, h1t, b1T_s)
                    agout1_prev = emit_ag1(h1t)

                # ---- FC(t-2) on R1 = h1T_full(t-2) -----------------------
                if t >= 2:
                    tau = t - 2
                    for j0 in range(0, 1024, 512):
                        w = min(512, Vc - j0)
                        fps = fcps.tile([P, 512], F32, tag="fc")
                        for k in range(KH):
                            nc.tensor.matmul(
                                fps[:, :w],
                                R1[:, k, :],
                                fcw_s[:, k, j0 : j0 + w],
                                start=(k == 0),
                                stop=(k == KH - 1),
                            )
                        ot = fcoutp.tile([P, 512], F32, tag="ot")
                        nc.vector.tensor_add(
                            ot[:, :w], fps[:, :w], fcb_s[:, j0 : j0 + w]
                        )
                        nc.scalar.dma_start(
                            out[tau * P : (tau + 1) * P, j0 : j0 + w],
                            ot[:, :w],
                        )

                # ---- A-compute filler for a later chunk ------------------
                if t + 4 < T:
                    a_compute(t + 4)

                # ---- FC tail chunk ---------------------------------------
                if t >= 2:
                    tau = t - 2
                    j0 = 1024
                    w = Vc - j0
                    fps = fcps.tile([P, 512], F32, tag="fc")
                    for k in range(KH):
                        nc.tensor.matmul(
                            fps[:, :w],
                            R1[:, k, :],
                            fcw_s[:, k, j0 : j0 + w],
                            start=(k == 0),
                            stop=(k == KH - 1),
                        )
                    ot = fcoutp.tile([P, 512], F32, tag="ot")
                    nc.vector.tensor_add(
                        ot[:, :w], fps[:, :w], fcb_s[:, j0 : j0 + w]
                    )
                    nc.scalar.dma_start(
                        out[tau * P : (tau + 1) * P, j0 : j0 + w],
                        ot[:, :w],
                    )

            for pool in (embcp, x0ps, embps, initwp, featp, fcps,
                         g1ps, g0ps, agout1p, agout0p, agin1p, agin0p,
                         fcoutp, gactp, h1p, h0p, rb1p, rb0p, rowsp, x0p,
                         statep, wresp, constp):
                pool.release()

    nc.finalize()
    return nc


def _get_compiled():
    if "nc" not in _cache:
        _cache["nc"] = _build_nc()
    return _cache["nc"]


def _prep_inputs(features, captions, embed_table, init_h_w, init_h_b,
                 init_c_w, init_c_b, w_ih0, w_hh0, b_ih0, b_hh0,
                 w_ih1, w_hh1, b_ih1, b_hh1, fc_w, fc_b):
    f16 = lambda x: np.ascontiguousarray(np.asarray(x, dtype=np.float32)).astype(np.float16)
    f32 = lambda x: np.ascontiguousarray(np.asarray(x, dtype=np.float32))

    features = np.asarray(features, dtype=np.float32)
    captions = np.asarray(captions).astype(np.int32)

    shared = {
        "featT": f16(features.T),
        "table": f16(embed_table),
        # row r = t*B + b  ->  captions[b, t]
        "emb_idx": np.ascontiguousarray(captions.T.reshape(TB, 1)),
    }

    # torch gate order i,f,g,o -> local order [i, g, f, o]
    def gate_rows(c):
        base = np.arange(c * Hc, (c + 1) * Hc)
        return np.concatenate([base, 2 * H + base, H + base, 3 * H + base])

    def init_sel(c):
        # Linear output col r maps to (h = r // L, l = r % L)
        h_idx = np.arange(c * Hc, (c + 1) * Hc)
        return 2 * h_idx, 2 * h_idx + 1   # l0 rows, l1 rows

    in_maps = []
    for c in range(NCORES):
        rows_sel = gate_rows(c)
        l0, l1 = init_sel(c)
        ihw = np.asarray(init_h_w, np.float32)
        icw = np.asarray(init_c_w, np.float32)
        ihb = np.asarray(init_h_b, np.float32)
        icb = np.asarray(init_c_b, np.float32)
        initw = np.concatenate([ihw[l0], ihw[l1], icw[l0], icw[l1]], axis=0)
        initb = np.concatenate([ihb[l0], ihb[l1], icb[l0], icb[l1]])

        b0 = (np.asarray(b_ih0, np.float32) + np.asarray(b_hh0, np.float32))[rows_sel]
        b1 = (np.asarray(b_ih1, np.float32) + np.asarray(b_hh1, np.float32))[rows_sel]

        vsl = slice(c * Vc, (c + 1) * Vc)
        m = dict(shared)
        m.update({
            "initw": f16(initw.T),
            "initbT": f32(initb.reshape(4, P).T),
            "wih0T": f16(np.asarray(w_ih0, np.float32)[rows_sel].T),
            "whh0T": f16(np.asarray(w_hh0, np.float32)[rows_sel].T),
            "wih1T": f16(np.asarray(w_ih1, np.float32)[rows_sel].T),
            "whh1T": f16(np.asarray(w_hh1, np.float32)[rows_sel].T),
            "b0T": f32(b0.reshape(4, P).T),
            "b1T": f32(b1.reshape(4, P).T),
            "fcwT": f16(np.asarray(fc_w, np.float32)[vsl].T),
            "fcb_rep": f32(np.broadcast_to(
                np.asarray(fc_b, np.float32)[vsl], (P, Vc))),
        })
        in_maps.append(m)
    return in_maps


last_results = None


def kernel(**inputs) -> np.ndarray:
    global last_results
    nc = _get_compiled()
    in_maps = _prep_inputs(**inputs)
    res = run_bass_kernel_spmd(nc, in_maps, core_ids=list(range(NCORES)))
    last_results = res
    parts = [res.results[c]["out"].reshape(T, B, Vc) for c in range(NCORES)]
    return np.concatenate(parts, axis=2)
